# revision 4
# baseline (speedup 1.0000x reference)
"""BinaryTreeLSTM on 8 Trainium2 NeuronCores.

Strategy: data-parallel over the leaf batch. Core d owns leaves
[1024*d, 1024*(d+1)) and folds its subtree through 10 merge levels
(1024 -> 1 node). The 8 per-core roots are AllGathered and the final
3 levels (8 -> 1) run replicated on every core.

Layout: feature-major. State tensors c,h live in SBUF as
[128 partitions, 2*B] where column (chunk*B + n) holds features
[chunk*128, chunk*128+128) of node n. Child selection (even/odd nodes)
is then a stride-2 slice along the free dim, and the gate projections
are K-chunked matmuls with the weights stationary:
    g[1280, B] = Wl @ lh + Wr @ rh + pad_xg  (pad via a K=1 rank-1 matmul)
"""

import numpy as np

IN_DIM = 300
MEM_DIM = 256
N_LEAVES = 8192
N_CORES = 8
LPC = N_LEAVES // N_CORES  # 1024 leaves per core

# px offsets: internal-gate m-chunk (5-gate [u,i,lf,rf,o] layout, 10 chunks
# of 128 rows) -> offset into the 4-gate [cx,ix,fx,ox] pad vector (lf and rf
# both read fx).
_PX_OFF = [0, 128, 256, 384, 512, 640, 512, 640, 768, 896]

_CACHE = {}


def _build():
    import concourse.bacc as bacc
    import concourse.mybir as mybir
    import concourse.tile as tile

    f32 = mybir.dt.float32
    AF = mybir.ActivationFunctionType

    nc = bacc.Bacc("TRN2", target_bir_lowering=False, debug=False,
                   num_devices=N_CORES)

    embsT = nc.dram_tensor("embsT", [IN_DIM, LPC], f32, kind="ExternalInput").ap()
    WxT = nc.dram_tensor("WxT", [IN_DIM, 1024], f32, kind="ExternalInput").ap()
    WlT = nc.dram_tensor("WlT", [MEM_DIM, 1280], f32, kind="ExternalInput").ap()
    WrT = nc.dram_tensor("WrT", [MEM_DIM, 1280], f32, kind="ExternalInput").ap()
    bxr = nc.dram_tensor("bxr", [1, 1024], f32, kind="ExternalInput").ap()
    padT = nc.dram_tensor("padT", [IN_DIM, 1], f32, kind="ExternalInput").ap()
    eye8 = nc.dram_tensor("eye8", [8, 8], f32, kind="ExternalInput").ap()
    out = nc.dram_tensor("out", [2, 1, MEM_DIM], f32, kind="ExternalOutput").ap()

    with tile.TileContext(nc) as tc:
        with (
            tc.tile_pool(name="const", bufs=1) as const,
            tc.tile_pool(name="state", bufs=2) as state,
            tc.tile_pool(name="gates", bufs=2) as gates,
            tc.tile_pool(name="psum", bufs=3, space="PSUM") as psum,
            tc.tile_pool(name="dram", bufs=1, space="DRAM") as dram,
        ):
            # ---- constants into SBUF ----
            # chunked layouts: [128, n_chunks * width], chunk k at cols
            # [k*width, (k+1)*width)
            WxT_sb = const.tile([128, 3 * 1024], f32)
            for k in range(3):
                r = 128 if k < 2 else IN_DIM - 256
                nc.sync.dma_start(WxT_sb[0:r, k * 1024:(k + 1) * 1024],
                                  WxT[128 * k:128 * k + r, :])
            embsT_sb = const.tile([128, 3 * LPC], f32)
            for k in range(3):
                r = 128 if k < 2 else IN_DIM - 256
                nc.sync.dma_start(embsT_sb[0:r, k * LPC:(k + 1) * LPC],
                                  embsT[128 * k:128 * k + r, :])
            WlT_sb = const.tile([128, 2 * 1280], f32)
            WrT_sb = const.tile([128, 2 * 1280], f32)
            for k in range(2):
                nc.sync.dma_start(WlT_sb[:, k * 1280:(k + 1) * 1280],
                                  WlT[128 * k:128 * (k + 1), :])
                nc.sync.dma_start(WrT_sb[:, k * 1280:(k + 1) * 1280],
                                  WrT[128 * k:128 * (k + 1), :])
            bx_sb = const.tile([1, 1024], f32)
            nc.sync.dma_start(bx_sb[:, :], bxr[:, :])
            padT_sb = const.tile([128, 3], f32)
            for k in range(3):
                r = 128 if k < 2 else IN_DIM - 256
                nc.sync.dma_start(padT_sb[0:r, k:k + 1], padT[128 * k:128 * k + r, :])
            eye_sb = const.tile([8, 8], f32)
            nc.sync.dma_start(eye_sb[:, :], eye8[:, :])
            ones_sb = const.tile([1, 512], f32)
            nc.vector.memset(ones_sb[:, :], 1.0)

            # ---- px = pad_row @ Wx.T + bx, node-major [1, 1024] ----
            px_ps = psum.tile([1, 1024], f32, tag="g")
            for nh in range(2):
                for k in range(3):
                    r = 128 if k < 2 else IN_DIM - 256
                    nc.tensor.matmul(
                        px_ps[:, nh * 512:(nh + 1) * 512],
                        padT_sb[0:r, k:k + 1],
                        WxT_sb[0:r, k * 1024 + nh * 512:k * 1024 + (nh + 1) * 512],
                        start=(k == 0), stop=(k == 2))
            px_sb = const.tile([1, 1024], f32)
            nc.vector.tensor_add(px_sb[:, :], px_ps[:, :], bx_sb[:, :])

            # ---- leaf phase: xg = embs @ Wx.T + bx; c = i*u; h = o*tanh(c) ----
            c_cur = state.tile([128, 2 * LPC], f32, tag="c")
            h_cur = state.tile([128, 2 * LPC], f32, tag="h")
            c3 = c_cur.rearrange("p (c n) -> p c n", c=2)
            h3 = h_cur.rearrange("p (c n) -> p c n", c=2)
            GL = 512  # leaves per supergroup
            for sg in range(LPC // GL):
                xg = {}
                for gname, gm in (("u", 0), ("i", 1), ("o", 3)):
                    t = psum.tile([128, 2 * GL], f32, tag="g", name=f"x{gname}{sg}")
                    for half in range(2):
                        m = gm * 2 + half
                        dst = t[:, half * GL:(half + 1) * GL]
                        for ki in range(4):
                            if ki < 3:
                                r = 128 if ki < 2 else IN_DIM - 256
                                lhsT = WxT_sb[0:r, ki * 1024 + m * 128:
                                              ki * 1024 + (m + 1) * 128]
                                rhs = embsT_sb[0:r, ki * LPC + sg * GL:
                                               ki * LPC + (sg + 1) * GL]
                            else:
                                lhsT = bx_sb[0:1, m * 128:(m + 1) * 128]
                                rhs = ones_sb[0:1, 0:GL]
                            nc.tensor.matmul(dst, lhsT, rhs,
                                             start=(ki == 0), stop=(ki == 3))
                    xg[gname] = t
                ut = gates.tile([128, 2 * GL], f32, tag="u", name=f"u{sg}")
                it = gates.tile([128, 2 * GL], f32, tag="i", name=f"i{sg}")
                ot = gates.tile([128, 2 * GL], f32, tag="o", name=f"o{sg}")
                tht = gates.tile([128, 2 * GL], f32, tag="th", name=f"th{sg}")
                nc.scalar.activation(ut[:, :], xg["u"][:, :], AF.Tanh)
                nc.scalar.activation(it[:, :], xg["i"][:, :], AF.Sigmoid)
                nc.scalar.activation(ot[:, :], xg["o"][:, :], AF.Sigmoid)
                cs = c3[:, :, sg * GL:(sg + 1) * GL]
                hs = h3[:, :, sg * GL:(sg + 1) * GL]
                u3 = ut.rearrange("p (c n) -> p c n", c=2)
                i3 = it.rearrange("p (c n) -> p c n", c=2)
                o3 = ot.rearrange("p (c n) -> p c n", c=2)
                th3 = tht.rearrange("p (c n) -> p c n", c=2)
                nc.vector.tensor_mul(cs, i3, u3)
                nc.scalar.activation(th3, cs, AF.Tanh)
                nc.vector.tensor_mul(hs, o3, th3)

            # ---- generic merge level ----
            def level(cp, hp, Bp, lvl):
                B = Bp // 2
                cn = state.tile([128, 2 * B], f32, tag="c", name=f"c{lvl}")
                hn = state.tile([128, 2 * B], f32, tag="h", name=f"h{lvl}")
                cp3 = cp.rearrange("p (c n) -> p c n", c=2)
                hp3 = hp.rearrange("p (c n) -> p c n", c=2)
                cn3 = cn.rearrange("p (c n) -> p c n", c=2)
                hn3 = hn.rearrange("p (c n) -> p c n", c=2)
                for g0 in range(0, B, 256):
                    G = min(256, B - g0)
                    gt = []
                    for gi in range(5):
                        t = psum.tile([128, 2 * G], f32, tag="g",
                                      name=f"g{lvl}_{g0}_{gi}")
                        for half in range(2):
                            m = gi * 2 + half
                            dst = t[:, half * G:(half + 1) * G]
                            for ki in range(5):
                                if ki < 4:
                                    W = WlT_sb if ki < 2 else WrT_sb
                                    kc = ki % 2
                                    lhsT = W[:, kc * 1280 + m * 128:
                                             kc * 1280 + (m + 1) * 128]
                                    par = 0 if ki < 2 else 1
                                    src = hp3[:, kc,
                                              2 * g0 + par:2 * (g0 + G):2]
                                    rhs = src
                                else:
                                    lhsT = px_sb[0:1, _PX_OFF[m]:_PX_OFF[m] + 128]
                                    rhs = ones_sb[0:1, 0:G]
                                nc.tensor.matmul(dst, lhsT, rhs,
                                                 start=(ki == 0), stop=(ki == 4))
                        gt.append(t)
                    sfx = f"{lvl}_{g0}"
                    ut = gates.tile([128, 2 * G], f32, tag="u", name=f"u{sfx}")
                    it = gates.tile([128, 2 * G], f32, tag="i", name=f"i{sfx}")
                    lft = gates.tile([128, 2 * G], f32, tag="lf", name=f"lf{sfx}")
                    rft = gates.tile([128, 2 * G], f32, tag="rf", name=f"rf{sfx}")
                    ot = gates.tile([128, 2 * G], f32, tag="o", name=f"o{sfx}")
                    tht = gates.tile([128, 2 * G], f32, tag="th", name=f"th{sfx}")
                    x1 = gates.tile([128, 2 * G], f32, tag="x1", name=f"x1{sfx}")
                    x2 = gates.tile([128, 2 * G], f32, tag="x2", name=f"x2{sfx}")
                    x3 = gates.tile([128, 2 * G], f32, tag="x3", name=f"x3{sfx}")
                    s1 = gates.tile([128, 2 * G], f32, tag="s1", name=f"s1{sfx}")
                    nc.scalar.activation(ut[:, :], gt[0][:, :], AF.Tanh)
                    nc.scalar.activation(it[:, :], gt[1][:, :], AF.Sigmoid)
                    nc.scalar.activation(lft[:, :], gt[2][:, :], AF.Sigmoid)
                    nc.scalar.activation(rft[:, :], gt[3][:, :], AF.Sigmoid)
                    nc.scalar.activation(ot[:, :], gt[4][:, :], AF.Sigmoid)
                    lc = cp3[:, :, 2 * g0:2 * (g0 + G):2]
                    rc = cp3[:, :, 2 * g0 + 1:2 * (g0 + G):2]
                    v = lambda t: t.rearrange("p (c n) -> p c n", c=2)
                    nc.vector.tensor_mul(v(x1), v(it), v(ut))
                    nc.vector.tensor_mul(v(x2), v(lft), lc)
                    nc.vector.tensor_mul(v(x3), v(rft), rc)
                    nc.vector.tensor_add(v(s1), v(x1), v(x2))
                    cs = cn3[:, :, g0:g0 + G]
                    hs = hn3[:, :, g0:g0 + G]
                    nc.vector.tensor_add(cs, v(s1), v(x3))
                    nc.scalar.activation(v(tht), cs, AF.Tanh)
                    nc.vector.tensor_mul(hs, v(ot), v(tht))
                return cn, hn, B

            Bp = LPC
            for lvl in range(10):
                c_cur, h_cur, Bp = level(c_cur, h_cur, Bp, lvl)

            # ---- AllGather the 8 per-core roots ----
            cc_in = dram.tile([1, 512], f32)
            cc_out = dram.tile([8, 512], f32, addr_space="Shared")
            nc.sync.dma_start(cc_in[0:1, 0:128], c_cur[:, 0:1])
            nc.sync.dma_start(cc_in[0:1, 128:256], c_cur[:, 1:2])
            nc.sync.dma_start(cc_in[0:1, 256:384], h_cur[:, 0:1])
            nc.sync.dma_start(cc_in[0:1, 384:512], h_cur[:, 1:2])
            nc.gpsimd.collective_compute(
                "AllGather",
                mybir.AluOpType.bypass,
                replica_groups=[list(range(N_CORES))],
                ins=[cc_in.opt()],
                outs=[cc_out.opt()],
            )
            roots_sb = const.tile([8, 512], f32)
            nc.sync.dma_start(roots_sb[:, :], cc_out[:, :])

            # transpose [8 nodes, 512] node-major -> feature-major [128, 2*8]
            c_cur = state.tile([128, 16], f32, tag="c", name="c_ag")
            h_cur = state.tile([128, 16], f32, tag="h", name="h_ag")
            for dst, base in ((c_cur, 0), (h_cur, 256)):
                for half in range(2):
                    tp = psum.tile([128, 8], f32, tag="tp", bufs=2,
                                   name=f"tp{base}_{half}")
                    nc.tensor.transpose(
                        tp[:, :],
                        roots_sb[0:8, base + half * 128:base + (half + 1) * 128],
                        eye_sb[0:8, 0:8])
                    nc.vector.tensor_copy(dst[:, half * 8:(half + 1) * 8], tp[:, :])

            # ---- final 3 levels, replicated ----
            Bp = 8
            for lvl in range(10, 13):
                c_cur, h_cur, Bp = level(c_cur, h_cur, Bp, lvl)

            # ---- write root (c, h) ----
            nc.sync.dma_start(out[0:1, 0:1, 0:128], c_cur[:, 0:1])
            nc.sync.dma_start(out[0:1, 0:1, 128:256], c_cur[:, 1:2])
            nc.sync.dma_start(out[1:2, 0:1, 0:128], h_cur[:, 0:1])
            nc.sync.dma_start(out[1:2, 0:1, 128:256], h_cur[:, 1:2])

    nc.compile()
    return nc


def _get_nc():
    if "nc" not in _CACHE:
        _CACHE["nc"] = _build()
    return _CACHE["nc"]


def kernel(embs, Wx, bx, Wl, Wr, emb_table, _trace=False, _trace_kwargs=None):
    from concourse.bass_utils import run_bass_kernel_spmd

    embs = np.ascontiguousarray(np.asarray(embs, dtype=np.float32))
    Wx = np.asarray(Wx, dtype=np.float32)
    bx = np.asarray(bx, dtype=np.float32)
    Wl = np.asarray(Wl, dtype=np.float32)
    Wr = np.asarray(Wr, dtype=np.float32)
    emb_table = np.asarray(emb_table, dtype=np.float32)

    WxT = np.ascontiguousarray(Wx.T)
    WlT = np.ascontiguousarray(Wl.T)
    WrT = np.ascontiguousarray(Wr.T)
    bxr = np.ascontiguousarray(bx.reshape(1, 1024))
    padT = np.ascontiguousarray(emb_table[-1].reshape(IN_DIM, 1))
    eye8 = np.eye(8, dtype=np.float32)

    in_maps = []
    for d in range(N_CORES):
        shard = np.ascontiguousarray(embs[d * LPC:(d + 1) * LPC].T)
        in_maps.append({
            "embsT": shard, "WxT": WxT, "WlT": WlT, "WrT": WrT,
            "bxr": bxr, "padT": padT, "eye8": eye8,
        })

    nc = _get_nc()
    res = run_bass_kernel_spmd(nc, in_maps, list(range(N_CORES)),
                               trace=_trace, **(_trace_kwargs or {}))
    _CACHE["last_result"] = res
    return np.asarray(res.results[0]["out"], dtype=np.float32)


# revision 11
# speedup vs baseline: 2.1316x; 2.1316x over previous
"""BinaryTreeLSTM on 8 Trainium2 NeuronCores.

Strategy: data-parallel over the leaf batch. Core d owns leaves
[1024*d, 1024*(d+1)) and folds its subtree through 10 merge levels
(1024 -> 1 node). The 8 per-core roots are AllGathered and the final
3 levels (8 -> 1) run replicated on every core.

Layout: feature-major. State tensors c,h live in SBUF as
[128 partitions, 2*B] where column (chunk*B + n) holds features
[chunk*128, chunk*128+128) of node n. Child selection (even/odd nodes)
is then a stride-2 slice along the free dim, and the gate projections
are K-chunked matmuls with the weights stationary:
    g[1280, B] = Wl @ lh + Wr @ rh        (fp32r, single-pass PE)
    gate = act(g + pad_xg)                (pad via ACT per-partition bias)
"""

import numpy as np

IN_DIM = 300
MEM_DIM = 256
N_LEAVES = 8192
N_CORES = 8
LPC = N_LEAVES // N_CORES  # 1024 leaves per core

# internal-gate m-chunk (5-gate [u,i,lf,rf,o] layout, 10 chunks of 128 rows)
# -> column of the [128, 8] feature-major pad_xg ([cx,ix,fx,ox]; lf and rf
# share fx).
_PXCOL = [0, 1, 2, 3, 4, 5, 4, 5, 6, 7]

_CACHE = {}


def _build():
    import concourse.bacc as bacc
    import concourse.mybir as mybir
    import concourse.tile as tile

    f32 = mybir.dt.float32
    f32r = mybir.dt.float32r
    AF = mybir.ActivationFunctionType

    nc = bacc.Bacc("TRN2", target_bir_lowering=False, debug=False,
                   num_devices=N_CORES)

    embsT = nc.dram_tensor("embsT", [IN_DIM, LPC], f32r, kind="ExternalInput").ap()
    WxT = nc.dram_tensor("WxT", [IN_DIM, 1024], f32r, kind="ExternalInput").ap()
    WlT = nc.dram_tensor("WlT", [MEM_DIM, 1280], f32r, kind="ExternalInput").ap()
    WrT = nc.dram_tensor("WrT", [MEM_DIM, 1280], f32r, kind="ExternalInput").ap()
    bxr = nc.dram_tensor("bxr", [1, 1024], f32, kind="ExternalInput").ap()
    padT = nc.dram_tensor("padT", [IN_DIM, 1], f32r, kind="ExternalInput").ap()
    eye8 = nc.dram_tensor("eye8", [8, 8], f32, kind="ExternalInput").ap()
    out = nc.dram_tensor("out", [2, 1, MEM_DIM], f32, kind="ExternalOutput").ap()

    with tile.TileContext(nc) as tc:
        with (
            tc.tile_pool(name="const", bufs=1) as const,
            tc.tile_pool(name="state", bufs=2) as state,
            tc.tile_pool(name="gates", bufs=2) as gates,
            tc.tile_pool(name="psum", bufs=3, space="PSUM") as psum,
            tc.tile_pool(name="dram", bufs=1, space="DRAM") as dram,
        ):
            # ---- constants into SBUF ----
            # chunked layouts: [128, n_chunks * width], chunk k at cols
            # [k*width, (k+1)*width)
            WxT_sb = const.tile([128, 3 * 1024], f32r)
            for k in range(3):
                r = 128 if k < 2 else IN_DIM - 256
                nc.sync.dma_start(WxT_sb[0:r, k * 1024:(k + 1) * 1024],
                                  WxT[128 * k:128 * k + r, :])
            embsT_sb = const.tile([128, 3 * LPC], f32r)
            for k in range(3):
                r = 128 if k < 2 else IN_DIM - 256
                nc.sync.dma_start(embsT_sb[0:r, k * LPC:(k + 1) * LPC],
                                  embsT[128 * k:128 * k + r, :])
            WlT_sb = const.tile([128, 2 * 1280], f32r)
            WrT_sb = const.tile([128, 2 * 1280], f32r)
            for k in range(2):
                nc.sync.dma_start(WlT_sb[:, k * 1280:(k + 1) * 1280],
                                  WlT[128 * k:128 * (k + 1), :])
                nc.sync.dma_start(WrT_sb[:, k * 1280:(k + 1) * 1280],
                                  WrT[128 * k:128 * (k + 1), :])
            bx_sb = const.tile([1, 1024], f32)
            nc.sync.dma_start(bx_sb[:, :], bxr[:, :])
            # feature-major bias: column m holds bx[128m : 128(m+1)]
            bx_fm = const.tile([128, 8], f32)
            nc.sync.dma_start(bx_fm[:, :],
                              bxr.rearrange("o (m p) -> p (o m)", p=128))
            padT_sb = const.tile([128, 3], f32r)
            for k in range(3):
                r = 128 if k < 2 else IN_DIM - 256
                nc.sync.dma_start(padT_sb[0:r, k:k + 1], padT[128 * k:128 * k + r, :])
            eye_sb = const.tile([8, 8], f32)
            nc.sync.dma_start(eye_sb[:, :], eye8[:, :])

            # ---- px = pad_row @ Wx.T + bx, node-major [1, 1024] ----
            px_ps = psum.tile([1, 1024], f32, tag="g")
            for nh in range(2):
                for k in range(3):
                    r = 128 if k < 2 else IN_DIM - 256
                    nc.tensor.matmul(
                        px_ps[:, nh * 512:(nh + 1) * 512],
                        padT_sb[0:r, k:k + 1],
                        WxT_sb[0:r, k * 1024 + nh * 512:k * 1024 + (nh + 1) * 512],
                        start=(k == 0), stop=(k == 2))
            px_sb = const.tile([1, 1024], f32)
            nc.vector.tensor_add(px_sb[:, :], px_ps[:, :], bx_sb[:, :])
            # transpose px into feature-major [128, 8] (col m = chunk m)
            px_fm = const.tile([128, 8], f32)
            for m in range(8):
                tp = psum.tile([128, 1], f32, tag="tp", bufs=2, name=f"pxt{m}")
                nc.tensor.transpose(tp[:, :], px_sb[0:1, m * 128:(m + 1) * 128],
                                    eye_sb[0:1, 0:1])
                nc.scalar.copy(px_fm[:, m:m + 1], tp[:, :])

            # ---- leaf phase: xg = embs @ Wx.T + bx; c = i*u; h = o*tanh(c) ----
            c_cur = state.tile([128, 2 * LPC], f32, tag="c")
            h_cur = state.tile([128, 2 * LPC], f32r, tag="h")
            c3 = c_cur.rearrange("p (c n) -> p c n", c=2)
            h3 = h_cur.rearrange("p (c n) -> p c n", c=2)
            GL = 512  # leaves per supergroup
            for sg in range(LPC // GL):
                xg = {}
                for gname, gm in (("u", 0), ("i", 1), ("o", 3)):
                    t = psum.tile([128, 2 * GL], f32, tag="g", name=f"x{gname}{sg}")
                    for half in range(2):
                        m = gm * 2 + half
                        dst = t[:, half * GL:(half + 1) * GL]
                        for ki in range(3):
                            r = 128 if ki < 2 else IN_DIM - 256
                            lhsT = WxT_sb[0:r, ki * 1024 + m * 128:
                                          ki * 1024 + (m + 1) * 128]
                            rhs = embsT_sb[0:r, ki * LPC + sg * GL:
                                           ki * LPC + (sg + 1) * GL]
                            nc.tensor.matmul(dst, lhsT, rhs,
                                             start=(ki == 0), stop=(ki == 2))
                    xg[gname] = t
                ut = gates.tile([128, 2 * GL], f32, tag="u", name=f"u{sg}")
                it = gates.tile([128, 2 * GL], f32, tag="i", name=f"i{sg}")
                ot = gates.tile([128, 2 * GL], f32, tag="o", name=f"o{sg}")
                tht = gates.tile([128, 2 * GL], f32, tag="th", name=f"th{sg}")
                for gname, dst, fn, gm in (("u", ut, AF.Tanh, 0),
                                           ("i", it, AF.Sigmoid, 1),
                                           ("o", ot, AF.Sigmoid, 3)):
                    for half in range(2):
                        nc.scalar.activation(
                            dst[:, half * GL:(half + 1) * GL],
                            xg[gname][:, half * GL:(half + 1) * GL],
                            fn, bias=bx_fm[:, gm * 2 + half:gm * 2 + half + 1])
                cs = c3[:, :, sg * GL:(sg + 1) * GL]
                hs = h3[:, :, sg * GL:(sg + 1) * GL]
                u3 = ut.rearrange("p (c n) -> p c n", c=2)
                i3 = it.rearrange("p (c n) -> p c n", c=2)
                o3 = ot.rearrange("p (c n) -> p c n", c=2)
                th3 = tht.rearrange("p (c n) -> p c n", c=2)
                nc.vector.tensor_mul(cs, i3, u3)
                nc.scalar.activation(th3, cs, AF.Tanh)
                nc.vector.tensor_mul(hs, o3, th3)

            # ---- generic merge level ----
            # fp32r moving operands need free dim >= 2, so B=1 levels compute
            # a second junk column, and B=2 levels pad their state to 4
            # zeroed columns for the next level's stride-2 reads.
            def level(cp, hp, Bp, lvl, Bp_pad):
                B = Bp // 2
                Bpad = B if B >= 4 else (4 if B == 2 else 2)
                cn = state.tile([128, 2 * Bpad], f32, tag="c", name=f"c{lvl}")
                hn = state.tile([128, 2 * Bpad], f32r, tag="h", name=f"h{lvl}")
                cp3 = cp.rearrange("p (c n) -> p c n", c=2)
                hp3 = hp.rearrange("p (c n) -> p c n", c=2)
                cn3 = cn.rearrange("p (c n) -> p c n", c=2)
                hn3 = hn.rearrange("p (c n) -> p c n", c=2)
                if Bpad > B:
                    npad = Bpad - B
                    nc.vector.memset(cn3[:, :, B:Bpad], 0.0)
                    # memset can't write f32r; multiply-by-zero through DVE can
                    nc.vector.tensor_scalar_mul(
                        hn3[:, :, B:Bpad],
                        bx_fm[:, 0:2 * npad].rearrange("p (c n) -> p c n", c=2),
                        0.0)
                for g0 in range(0, B, 256):
                    G = max(min(256, B - g0), 2)
                    gt = []
                    for gi in range(5):
                        t = psum.tile([128, 2 * G], f32, tag="g",
                                      name=f"g{lvl}_{g0}_{gi}")
                        for half in range(2):
                            m = gi * 2 + half
                            dst = t[:, half * G:(half + 1) * G]
                            for ki in range(4):
                                W = WlT_sb if ki < 2 else WrT_sb
                                kc = ki % 2
                                lhsT = W[:, kc * 1280 + m * 128:
                                         kc * 1280 + (m + 1) * 128]
                                par = 0 if ki < 2 else 1
                                rhs = hp3[:, kc, 2 * g0 + par:2 * (g0 + G):2]
                                nc.tensor.matmul(dst, lhsT, rhs,
                                                 start=(ki == 0), stop=(ki == 3))
                        gt.append(t)
                    sfx = f"{lvl}_{g0}"
                    ut = gates.tile([128, 2 * G], f32, tag="u", name=f"u{sfx}")
                    it = gates.tile([128, 2 * G], f32, tag="i", name=f"i{sfx}")
                    lft = gates.tile([128, 2 * G], f32, tag="lf", name=f"lf{sfx}")
                    rft = gates.tile([128, 2 * G], f32, tag="rf", name=f"rf{sfx}")
                    ot = gates.tile([128, 2 * G], f32, tag="o", name=f"o{sfx}")
                    tht = gates.tile([128, 2 * G], f32, tag="th", name=f"th{sfx}")
                    x1 = gates.tile([128, 2 * G], f32, tag="x1", name=f"x1{sfx}")
                    x2 = gates.tile([128, 2 * G], f32, tag="x2", name=f"x2{sfx}")
                    x3 = gates.tile([128, 2 * G], f32, tag="x3", name=f"x3{sfx}")
                    s1 = gates.tile([128, 2 * G], f32, tag="s1", name=f"s1{sfx}")
                    for gi, (dst, fn) in enumerate((
                            (ut, AF.Tanh), (it, AF.Sigmoid), (lft, AF.Sigmoid),
                            (rft, AF.Sigmoid), (ot, AF.Sigmoid))):
                        for half in range(2):
                            m = gi * 2 + half
                            nc.scalar.activation(
                                dst[:, half * G:(half + 1) * G],
                                gt[gi][:, half * G:(half + 1) * G],
                                fn,
                                bias=px_fm[:, _PXCOL[m]:_PXCOL[m] + 1])
                    lc = cp3[:, :, 2 * g0:2 * (g0 + G):2]
                    rc = cp3[:, :, 2 * g0 + 1:2 * (g0 + G):2]
                    v = lambda t: t.rearrange("p (c n) -> p c n", c=2)
                    nc.vector.tensor_mul(v(x1), v(it), v(ut))
                    nc.vector.tensor_mul(v(x2), v(lft), lc)
                    nc.vector.tensor_mul(v(x3), v(rft), rc)
                    nc.vector.tensor_add(v(s1), v(x1), v(x2))
                    cs = cn3[:, :, g0:g0 + G]
                    hs = hn3[:, :, g0:g0 + G]
                    nc.vector.tensor_add(cs, v(s1), v(x3))
                    nc.scalar.activation(v(tht), cs, AF.Tanh)
                    nc.vector.tensor_mul(hs, v(ot), v(tht))
                return cn, hn, B, Bpad

            Bp = LPC
            Bp_pad = LPC
            for lvl in range(10):
                c_cur, h_cur, Bp, Bp_pad = level(c_cur, h_cur, Bp, lvl, Bp_pad)

            # ---- AllGather the 8 per-core roots ----
            Q = Bp_pad  # chunk stride of the root state tiles
            cc_in = dram.tile([1, 512], f32)
            cc_out = dram.tile([8, 512], f32, addr_space="Shared")
            nc.sync.dma_start(cc_in[0:1, 0:128], c_cur[:, 0:1])
            nc.sync.dma_start(cc_in[0:1, 128:256], c_cur[:, Q:Q + 1])
            nc.sync.dma_start(cc_in[0:1, 256:384], h_cur.bitcast(f32)[:, 0:1])
            nc.sync.dma_start(cc_in[0:1, 384:512], h_cur.bitcast(f32)[:, Q:Q + 1])
            nc.gpsimd.collective_compute(
                "AllGather",
                mybir.AluOpType.bypass,
                replica_groups=[list(range(N_CORES))],
                ins=[cc_in.opt()],
                outs=[cc_out.opt()],
            )
            roots_sb = const.tile([8, 512], f32)
            nc.sync.dma_start(roots_sb[:, :], cc_out[:, :])

            # transpose [8 nodes, 512] node-major -> feature-major [128, 2*8]
            c_cur = state.tile([128, 16], f32, tag="c", name="c_ag")
            h_cur = state.tile([128, 16], f32r, tag="h", name="h_ag")
            for dst, base in ((c_cur, 0), (h_cur, 256)):
                for half in range(2):
                    tp = psum.tile([128, 8], f32, tag="tp", bufs=2,
                                   name=f"tp{base}_{half}")
                    nc.tensor.transpose(
                        tp[:, :],
                        roots_sb[0:8, base + half * 128:base + (half + 1) * 128],
                        eye_sb[0:8, 0:8])
                    nc.vector.tensor_copy(dst[:, half * 8:(half + 1) * 8], tp[:, :])

            # ---- final 3 levels, replicated ----
            Bp = 8
            Bp_pad = 8
            for lvl in range(10, 13):
                c_cur, h_cur, Bp, Bp_pad = level(c_cur, h_cur, Bp, lvl, Bp_pad)

            # ---- write root (c, h) ----
            Q = Bp_pad
            nc.sync.dma_start(out[0:1, 0:1, 0:128], c_cur[:, 0:1])
            nc.sync.dma_start(out[0:1, 0:1, 128:256], c_cur[:, Q:Q + 1])
            nc.sync.dma_start(out[1:2, 0:1, 0:128], h_cur.bitcast(f32)[:, 0:1])
            nc.sync.dma_start(out[1:2, 0:1, 128:256], h_cur.bitcast(f32)[:, Q:Q + 1])

    nc.compile()
    return nc


def _get_nc():
    if "nc" not in _CACHE:
        _CACHE["nc"] = _build()
    return _CACHE["nc"]


def kernel(embs, Wx, bx, Wl, Wr, emb_table, _trace=False, _trace_kwargs=None):
    from concourse.bass_utils import run_bass_kernel_spmd

    embs = np.ascontiguousarray(np.asarray(embs, dtype=np.float32))
    Wx = np.asarray(Wx, dtype=np.float32)
    bx = np.asarray(bx, dtype=np.float32)
    Wl = np.asarray(Wl, dtype=np.float32)
    Wr = np.asarray(Wr, dtype=np.float32)
    emb_table = np.asarray(emb_table, dtype=np.float32)

    WxT = np.ascontiguousarray(Wx.T)
    WlT = np.ascontiguousarray(Wl.T)
    WrT = np.ascontiguousarray(Wr.T)
    bxr = np.ascontiguousarray(bx.reshape(1, 1024))
    padT = np.ascontiguousarray(emb_table[-1].reshape(IN_DIM, 1))
    eye8 = np.eye(8, dtype=np.float32)

    in_maps = []
    for d in range(N_CORES):
        shard = np.ascontiguousarray(embs[d * LPC:(d + 1) * LPC].T)
        in_maps.append({
            "embsT": shard, "WxT": WxT, "WlT": WlT, "WrT": WrT,
            "bxr": bxr, "padT": padT, "eye8": eye8,
        })

    nc = _get_nc()
    res = run_bass_kernel_spmd(nc, in_maps, list(range(N_CORES)),
                               trace=_trace, **(_trace_kwargs or {}))
    _CACHE["last_result"] = res
    return np.asarray(res.results[0]["out"], dtype=np.float32)


# revision 13
# speedup vs baseline: 2.3628x; 1.1085x over previous
"""BinaryTreeLSTM on 8 Trainium2 NeuronCores.

Data-parallel over the leaf batch: core d owns leaves [1024d, 1024d+1024)
and folds its subtree through 10 merge levels; the 8 per-core roots are
AllGathered and the final 3 levels run replicated on every core.

Two matmul regimes (fp32r operands, single-pass PE):
- Feature-major (leaf, B=512, B=256 levels): weights stationary, nodes
  on the moving free dim. State h is kept as [128, 2 chunks * B] with
  even/odd children split into separate tiles so weight loads and reads
  stay contiguous.
- Node-major (B <= 128 levels): h chunks stationary (tiny weight loads),
  W streams as the moving operand in 512-wide chunks. Gates/c/h are
  node-major [B, 256]; h is transposed back to feature-major via PE
  transposes for the next level, and lc/rc come from partition-strided
  SBUF DMAs of the previous node-major c.
"""

import numpy as np

IN_DIM = 300
MEM_DIM = 256
N_LEAVES = 8192
N_CORES = 8
LPC = N_LEAVES // N_CORES  # 1024 leaves per core

# FM-gate m-chunk (5-gate [u,i,lf,rf,o] x 2 halves) -> column of the
# [128, 8] feature-major pad_xg ([cx,ix,fx,ox]; lf and rf share fx)
_PXCOL = [0, 1, 2, 3, 4, 5, 4, 5, 6, 7]
# node-major 5-gate px layout offsets into the 4-gate [1,1024] px row
_PX5SRC = [0, 256, 512, 512, 768]

_CACHE = {}


def _build():
    import concourse.bacc as bacc
    import concourse.mybir as mybir
    import concourse.tile as tile

    f32 = mybir.dt.float32
    f32r = mybir.dt.float32r
    AF = mybir.ActivationFunctionType

    nc = bacc.Bacc("TRN2", target_bir_lowering=False, debug=False,
                   num_devices=N_CORES)

    embsT = nc.dram_tensor("embsT", [IN_DIM, LPC], f32r, kind="ExternalInput").ap()
    WxT = nc.dram_tensor("WxT", [IN_DIM, 1024], f32r, kind="ExternalInput").ap()
    WlT = nc.dram_tensor("WlT", [MEM_DIM, 1280], f32r, kind="ExternalInput").ap()
    WrT = nc.dram_tensor("WrT", [MEM_DIM, 1280], f32r, kind="ExternalInput").ap()
    bxr = nc.dram_tensor("bxr", [1, 1024], f32, kind="ExternalInput").ap()
    padT = nc.dram_tensor("padT", [IN_DIM, 1], f32r, kind="ExternalInput").ap()
    eye_in = nc.dram_tensor("eye_in", [128, 128], f32, kind="ExternalInput").ap()
    ones_in = nc.dram_tensor("ones_in", [1, 128], f32r, kind="ExternalInput").ap()
    out = nc.dram_tensor("out", [2, 1, MEM_DIM], f32, kind="ExternalOutput").ap()

    with tile.TileContext(nc) as tc:
        with (
            tc.tile_pool(name="const", bufs=1) as const,
            tc.tile_pool(name="state", bufs=2) as state,
            tc.tile_pool(name="gates", bufs=2) as gates,
            tc.tile_pool(name="psum", bufs=2, space="PSUM") as psum,
            tc.tile_pool(name="dram", bufs=1, space="DRAM") as dram,
        ):
            v2 = lambda t: t.rearrange("p (c n) -> p c n", c=2)

            # ---- constants ----
            WxT_sb = const.tile([128, 3 * 1024], f32r)
            embsT_sb = const.tile([128, 3 * LPC], f32r)
            for k in range(3):
                r = 128 if k < 2 else IN_DIM - 256
                nc.sync.dma_start(WxT_sb[0:r, k * 1024:(k + 1) * 1024],
                                  WxT[128 * k:128 * k + r, :])
                nc.sync.dma_start(embsT_sb[0:r, k * LPC:(k + 1) * LPC],
                                  embsT[128 * k:128 * k + r, :])
            WlT_sb = const.tile([128, 2 * 1280], f32r)
            WrT_sb = const.tile([128, 2 * 1280], f32r)
            for k in range(2):
                nc.sync.dma_start(WlT_sb[:, k * 1280:(k + 1) * 1280],
                                  WlT[128 * k:128 * (k + 1), :])
                nc.sync.dma_start(WrT_sb[:, k * 1280:(k + 1) * 1280],
                                  WrT[128 * k:128 * (k + 1), :])
            bx_sb = const.tile([1, 1024], f32)
            nc.sync.dma_start(bx_sb[:, :], bxr[:, :])
            bx_fm = const.tile([128, 8], f32)
            nc.sync.dma_start(bx_fm[:, :],
                              bxr.rearrange("o (m p) -> p (o m)", p=128))
            padT_sb = const.tile([128, 3], f32r)
            for k in range(3):
                r = 128 if k < 2 else IN_DIM - 256
                nc.sync.dma_start(padT_sb[0:r, k:k + 1], padT[128 * k:128 * k + r, :])
            eye_sb = const.tile([128, 128], f32)
            nc.sync.dma_start(eye_sb[:, :], eye_in[:, :])
            ones_sb = const.tile([1, 128], f32r)
            nc.sync.dma_start(ones_sb[:, :], ones_in[:, :])

            # ---- px = pad_row @ Wx.T + bx ----
            px_ps = psum.tile([1, 1024], f32, tag="g")
            for nh in range(2):
                for k in range(3):
                    r = 128 if k < 2 else IN_DIM - 256
                    nc.tensor.matmul(
                        px_ps[:, nh * 512:(nh + 1) * 512],
                        padT_sb[0:r, k:k + 1],
                        WxT_sb[0:r, k * 1024 + nh * 512:k * 1024 + (nh + 1) * 512],
                        start=(k == 0), stop=(k == 2))
            px_sb = const.tile([1, 1024], f32)
            nc.vector.tensor_add(px_sb[:, :], px_ps[:, :], bx_sb[:, :])
            px_fm = const.tile([128, 8], f32)
            for m in range(8):
                tp = psum.tile([128, 1], f32, tag="tp", name=f"pxt{m}")
                nc.tensor.transpose(tp[:, :], px_sb[0:1, m * 128:(m + 1) * 128],
                                    eye_sb[0:1, 0:1])
                nc.scalar.copy(px_fm[:, m:m + 1], tp[:, :])
            px5 = const.tile([1, 1280], f32r)  # node-major 5-gate pad row
            for g in range(5):
                nc.vector.tensor_copy(
                    px5[0:1, 256 * g:256 * (g + 1)],
                    px_sb[0:1, _PX5SRC[g]:_PX5SRC[g] + 256])

            # ---- leaf phase ----
            c0 = state.tile([128, 2 * LPC], f32, tag="c")
            hev = state.tile([128, 2 * 512], f32r, tag="hev", name="hev_leaf")
            hod = state.tile([128, 2 * 512], f32r, tag="hod", name="hod_leaf")
            c0_3, hev3, hod3 = v2(c0), v2(hev), v2(hod)
            GL = 512
            for sg in range(LPC // GL):
                xg = {}
                for gname, gm in (("u", 0), ("i", 1), ("o", 3)):
                    t = psum.tile([128, 2 * GL], f32, tag="g", name=f"x{gname}{sg}")
                    for half in range(2):
                        m = gm * 2 + half
                        dst = t[:, half * GL:(half + 1) * GL]
                        for ki in range(3):
                            r = 128 if ki < 2 else IN_DIM - 256
                            nc.tensor.matmul(
                                dst,
                                WxT_sb[0:r, ki * 1024 + m * 128:
                                       ki * 1024 + (m + 1) * 128],
                                embsT_sb[0:r, ki * LPC + sg * GL:
                                         ki * LPC + (sg + 1) * GL],
                                start=(ki == 0), stop=(ki == 2))
                    xg[gname] = t
                ut = gates.tile([128, 2 * GL], f32, tag="u", name=f"u{sg}")
                it = gates.tile([128, 2 * GL], f32, tag="i", name=f"i{sg}")
                ot = gates.tile([128, 2 * GL], f32, tag="o", name=f"o{sg}")
                tht = gates.tile([128, 2 * GL], f32, tag="th", name=f"th{sg}")
                for gname, dst, fn, gm in (("u", ut, AF.Tanh, 0),
                                           ("i", it, AF.Sigmoid, 1),
                                           ("o", ot, AF.Sigmoid, 3)):
                    for half in range(2):
                        nc.scalar.activation(
                            dst[:, half * GL:(half + 1) * GL],
                            xg[gname][:, half * GL:(half + 1) * GL],
                            fn, bias=bx_fm[:, gm * 2 + half:gm * 2 + half + 1])
                cs = c0_3[:, :, sg * GL:(sg + 1) * GL]
                u3, i3, o3, th3 = v2(ut), v2(it), v2(ot), v2(tht)
                nc.vector.tensor_mul(cs, i3, u3)
                nc.scalar.activation(th3, cs, AF.Tanh)
                nc.vector.tensor_mul(hev3[:, :, sg * 256:(sg + 1) * 256],
                                     o3[:, :, 0::2], th3[:, :, 0::2])
                nc.vector.tensor_mul(hod3[:, :, sg * 256:(sg + 1) * 256],
                                     o3[:, :, 1::2], th3[:, :, 1::2])

            # ---- feature-major level (B >= 256) ----
            def fm_level(cp, hev_p, hod_p, Bp, lvl, split_c):
                B = Bp // 2
                hev_n = state.tile([128, 2 * (B // 2)], f32r, tag="hev",
                                   name=f"hev{lvl}")
                hod_n = state.tile([128, 2 * (B // 2)], f32r, tag="hod",
                                   name=f"hod{lvl}")
                if split_c:
                    cev = state.tile([128, 2 * (B // 2)], f32, tag="cev",
                                     name=f"cev{lvl}", bufs=1)
                    cod = state.tile([128, 2 * (B // 2)], f32, tag="cod",
                                     name=f"cod{lvl}", bufs=1)
                else:
                    cn = state.tile([128, 2 * B], f32, tag="c", name=f"c{lvl}")
                cp3 = v2(cp)
                for g0 in range(0, B, 256):
                    G = min(256, B - g0)
                    gt = []
                    for gi in range(5):
                        t = psum.tile([128, 2 * G], f32, tag="g",
                                      name=f"g{lvl}_{g0}_{gi}")
                        for half in range(2):
                            m = gi * 2 + half
                            dst = t[:, half * G:(half + 1) * G]
                            for ki in range(4):
                                W = WlT_sb if ki < 2 else WrT_sb
                                kc = ki % 2
                                hp = hev_p if ki < 2 else hod_p
                                nc.tensor.matmul(
                                    dst,
                                    W[:, kc * 1280 + m * 128:
                                      kc * 1280 + (m + 1) * 128],
                                    v2(hp)[:, kc, g0:g0 + G],
                                    start=(ki == 0), stop=(ki == 3))
                        gt.append(t)
                    sfx = f"{lvl}_{g0}"
                    ut = gates.tile([128, 2 * G], f32, tag="u", name=f"u{sfx}")
                    it = gates.tile([128, 2 * G], f32, tag="i", name=f"i{sfx}")
                    lft = gates.tile([128, 2 * G], f32, tag="lf", name=f"lf{sfx}")
                    rft = gates.tile([128, 2 * G], f32, tag="rf", name=f"rf{sfx}")
                    ot = gates.tile([128, 2 * G], f32, tag="o", name=f"o{sfx}")
                    tht = gates.tile([128, 2 * G], f32, tag="th", name=f"th{sfx}")
                    x1 = gates.tile([128, 2 * G], f32, tag="x1", name=f"x1{sfx}")
                    x2 = gates.tile([128, 2 * G], f32, tag="x2", name=f"x2{sfx}")
                    x3 = gates.tile([128, 2 * G], f32, tag="x3", name=f"x3{sfx}")
                    s1 = gates.tile([128, 2 * G], f32, tag="s1", name=f"s1{sfx}")
                    for gi, (dst, fn) in enumerate((
                            (ut, AF.Tanh), (it, AF.Sigmoid), (lft, AF.Sigmoid),
                            (rft, AF.Sigmoid), (ot, AF.Sigmoid))):
                        for half in range(2):
                            m = gi * 2 + half
                            nc.scalar.activation(
                                dst[:, half * G:(half + 1) * G],
                                gt[gi][:, half * G:(half + 1) * G],
                                fn, bias=px_fm[:, _PXCOL[m]:_PXCOL[m] + 1])
                    lc = cp3[:, :, 2 * g0:2 * (g0 + G):2]
                    rc = cp3[:, :, 2 * g0 + 1:2 * (g0 + G):2]
                    u3, i3 = v2(ut), v2(it)
                    lf3, rf3, o3, th3 = v2(lft), v2(rft), v2(ot), v2(tht)
                    x13, x23, x33, s13 = v2(x1), v2(x2), v2(x3), v2(s1)
                    nc.vector.tensor_mul(x13, i3, u3)
                    nc.vector.tensor_mul(x23, lf3, lc)
                    nc.vector.tensor_mul(x33, rf3, rc)
                    nc.vector.tensor_add(s13, x13, x23)
                    if split_c:
                        ce = v2(cev)[:, :, g0 // 2:(g0 + G) // 2]
                        co = v2(cod)[:, :, g0 // 2:(g0 + G) // 2]
                        nc.vector.tensor_add(ce, s13[:, :, 0::2], x33[:, :, 0::2])
                        nc.vector.tensor_add(co, s13[:, :, 1::2], x33[:, :, 1::2])
                        nc.scalar.activation(th3[:, :, 0::2], ce, AF.Tanh)
                        nc.scalar.activation(th3[:, :, 1::2], co, AF.Tanh)
                    else:
                        cs = v2(cn)[:, :, g0:g0 + G]
                        nc.vector.tensor_add(cs, s13, x33)
                        nc.scalar.activation(th3, cs, AF.Tanh)
                    nc.vector.tensor_mul(v2(hev_n)[:, :, g0 // 2:(g0 + G) // 2],
                                         o3[:, :, 0::2], th3[:, :, 0::2])
                    nc.vector.tensor_mul(v2(hod_n)[:, :, g0 // 2:(g0 + G) // 2],
                                         o3[:, :, 1::2], th3[:, :, 1::2])
                if split_c:
                    return (cev, cod), hev_n, hod_n, B
                return cn, hev_n, hod_n, B

            # ---- node-major level (B <= 128) ----
            def nm_level(lc, rc, hev_p, hod_p, B, lvl, last):
                g_ps = psum.tile([128, 1280], f32, tag="g", name=f"gn{lvl}")
                for n0, nw in ((0, 512), (512, 512), (1024, 256)):
                    for ki in range(5):
                        if ki < 4:
                            par, kc = ki // 2, ki % 2
                            hsrc = hev_p if par == 0 else hod_p
                            lhsT = hsrc[:, kc * B:(kc + 1) * B]
                            W = WlT_sb if par == 0 else WrT_sb
                            rhs = W[:, kc * 1280 + n0:kc * 1280 + n0 + nw]
                        else:
                            lhsT = ones_sb[0:1, 0:B]
                            rhs = px5[0:1, n0:n0 + nw]
                        nc.tensor.matmul(g_ps[0:B, n0:n0 + nw], lhsT, rhs,
                                         start=(ki == 0), stop=(ki == 4))
                sfx = f"n{lvl}"
                ut = gates.tile([128, 256], f32, tag="u", name=f"u{sfx}")
                it = gates.tile([128, 256], f32, tag="i", name=f"i{sfx}")
                lft = gates.tile([128, 256], f32, tag="lf", name=f"lf{sfx}")
                rft = gates.tile([128, 256], f32, tag="rf", name=f"rf{sfx}")
                ot = gates.tile([128, 256], f32, tag="o", name=f"o{sfx}")
                tht = gates.tile([128, 256], f32, tag="th", name=f"th{sfx}")
                x1 = gates.tile([128, 256], f32, tag="x1", name=f"x1{sfx}")
                x2 = gates.tile([128, 256], f32, tag="x2", name=f"x2{sfx}")
                x3 = gates.tile([128, 256], f32, tag="x3", name=f"x3{sfx}")
                s1 = gates.tile([128, 256], f32, tag="s1", name=f"s1{sfx}")
                c_nm = state.tile([128, 256], f32, tag="cn", name=f"cn{lvl}")
                h_nm = state.tile([128, 256], f32, tag="hn", name=f"hn{lvl}")
                for gi, (dst, fn) in enumerate((
                        (ut, AF.Tanh), (it, AF.Sigmoid), (lft, AF.Sigmoid),
                        (rft, AF.Sigmoid), (ot, AF.Sigmoid))):
                    nc.scalar.activation(dst[0:B, :],
                                         g_ps[0:B, 256 * gi:256 * (gi + 1)], fn)
                nc.vector.tensor_mul(x1[0:B, :], it[0:B, :], ut[0:B, :])
                nc.vector.tensor_mul(x2[0:B, :], lft[0:B, :], lc)
                nc.vector.tensor_mul(x3[0:B, :], rft[0:B, :], rc)
                nc.vector.tensor_add(s1[0:B, :], x1[0:B, :], x2[0:B, :])
                nc.vector.tensor_add(c_nm[0:B, :], s1[0:B, :], x3[0:B, :])
                nc.scalar.activation(tht[0:B, :], c_nm[0:B, :], AF.Tanh)
                nc.vector.tensor_mul(h_nm[0:B, :], ot[0:B, :], tht[0:B, :])
                if last:
                    return c_nm, h_nm, None, None
                hev_n = state.tile([128, 2 * (B // 2)], f32r, tag="hev",
                                   name=f"hev{lvl}")
                hod_n = state.tile([128, 2 * (B // 2)], f32r, tag="hod",
                                   name=f"hod{lvl}")
                for kc in range(2):
                    tp = psum.tile([128, B], f32, tag="tp", name=f"tph{lvl}_{kc}")
                    nc.tensor.transpose(tp[:, :],
                                        h_nm[0:B, 128 * kc:128 * (kc + 1)],
                                        eye_sb[0:B, 0:B])
                    nc.vector.tensor_copy(
                        hev_n[:, kc * (B // 2):(kc + 1) * (B // 2)],
                        tp[:, 0:B:2])
                    nc.vector.tensor_copy(
                        hod_n[:, kc * (B // 2):(kc + 1) * (B // 2)],
                        tp[:, 1:B:2])
                return c_nm, h_nm, hev_n, hod_n

            def gather_children(c_src, B, lvl):
                # lc/rc = even/odd partitions of the node-major parent c
                lct = gates.tile([128, 256], f32, tag="lct", name=f"lct{lvl}")
                rct = gates.tile([128, 256], f32, tag="rct", name=f"rct{lvl}")
                nc.sync.dma_start(lct[0:B, :], c_src[0:2 * B:2, :])
                nc.sync.dma_start(rct[0:B, :], c_src[1:2 * B:2, :])
                return lct[0:B, :], rct[0:B, :]

            # lvl0 (1024->512, FM, contiguous c), lvl1 (512->256, FM, split c)
            c_lvl0, hev, hod, B = fm_level(c0, hev, hod, LPC, 0, False)
            (cev1, cod1), hev, hod, B = fm_level(c_lvl0, hev, hod, B, 1, True)

            # boundary: lc/rc for lvl2 via PE transposes of the split FM c
            lc2 = gates.tile([128, 256], f32, tag="lct", name="lc2")
            rc2 = gates.tile([128, 256], f32, tag="rct", name="rc2")
            for src, dst in ((cev1, lc2), (cod1, rc2)):
                for kc in range(2):
                    tp = psum.tile([128, 128], f32, tag="tp",
                                   name=f"tpb{dst.name}_{kc}")
                    nc.tensor.transpose(tp[:, :], v2(src)[:, kc, :],
                                        eye_sb[:, :])
                    nc.vector.tensor_copy(dst[:, 128 * kc:128 * (kc + 1)],
                                          tp[:, :])

            # lvl2..lvl9 node-major (B = 128..1)
            lc, rc = lc2[0:128, :], rc2[0:128, :]
            c_nm = None
            for lvl in range(2, 10):
                B >>= 1  # 128, 64, ..., 1
                last = (lvl == 9)
                c_nm, h_nm, hev, hod = nm_level(lc, rc, hev, hod, B, lvl, last)
                if not last:
                    lc, rc = gather_children(c_nm, B // 2, lvl + 1)

            # ---- AllGather the 8 per-core roots ----
            cc_in = dram.tile([1, 512], f32)
            cc_out = dram.tile([8, 512], f32, addr_space="Shared")
            nc.sync.dma_start(cc_in[0:1, 0:256], c_nm[0:1, :])
            nc.sync.dma_start(cc_in[0:1, 256:512], h_nm[0:1, :])
            nc.gpsimd.collective_compute(
                "AllGather",
                mybir.AluOpType.bypass,
                replica_groups=[list(range(N_CORES))],
                ins=[cc_in.opt()],
                outs=[cc_out.opt()],
            )
            roots_sb = const.tile([8, 512], f32)
            nc.sync.dma_start(roots_sb[:, :], cc_out[:, :])

            # prep lvl10 inputs from the gathered roots
            hev = state.tile([128, 2 * 4], f32r, tag="hev", name="hev_ag")
            hod = state.tile([128, 2 * 4], f32r, tag="hod", name="hod_ag")
            for kc in range(2):
                tp = psum.tile([128, 8], f32, tag="tp", name=f"tpag{kc}")
                nc.tensor.transpose(
                    tp[:, :], roots_sb[0:8, 256 + 128 * kc:256 + 128 * (kc + 1)],
                    eye_sb[0:8, 0:8])
                nc.vector.tensor_copy(hev[:, kc * 4:(kc + 1) * 4], tp[:, 0:8:2])
                nc.vector.tensor_copy(hod[:, kc * 4:(kc + 1) * 4], tp[:, 1:8:2])
            lct = gates.tile([128, 256], f32, tag="lct", name="lct10")
            rct = gates.tile([128, 256], f32, tag="rct", name="rct10")
            nc.sync.dma_start(lct[0:4, :], roots_sb[0:8:2, 0:256])
            nc.sync.dma_start(rct[0:4, :], roots_sb[1:8:2, 0:256])
            lc, rc = lct[0:4, :], rct[0:4, :]

            # final 3 levels, replicated (B = 4, 2, 1)
            B = 8
            for lvl in range(10, 13):
                B >>= 1
                last = (lvl == 12)
                c_nm, h_nm, hev, hod = nm_level(lc, rc, hev, hod, B, lvl, last)
                if not last:
                    lc, rc = gather_children(c_nm, B // 2, lvl + 1)

            # ---- write root (c, h) ----
            nc.sync.dma_start(out[0:1, 0:1, :], c_nm[0:1, :])
            nc.sync.dma_start(out[1:2, 0:1, :], h_nm[0:1, :])

    nc.compile()
    return nc


def _get_nc():
    if "nc" not in _CACHE:
        _CACHE["nc"] = _build()
    return _CACHE["nc"]


def kernel(embs, Wx, bx, Wl, Wr, emb_table, _trace=False, _trace_kwargs=None):
    from concourse.bass_utils import run_bass_kernel_spmd

    embs = np.ascontiguousarray(np.asarray(embs, dtype=np.float32))
    Wx = np.asarray(Wx, dtype=np.float32)
    bx = np.asarray(bx, dtype=np.float32)
    Wl = np.asarray(Wl, dtype=np.float32)
    Wr = np.asarray(Wr, dtype=np.float32)
    emb_table = np.asarray(emb_table, dtype=np.float32)

    WxT = np.ascontiguousarray(Wx.T)
    WlT = np.ascontiguousarray(Wl.T)
    WrT = np.ascontiguousarray(Wr.T)
    bxr = np.ascontiguousarray(bx.reshape(1, 1024))
    padT = np.ascontiguousarray(emb_table[-1].reshape(IN_DIM, 1))
    eye = np.eye(128, dtype=np.float32)
    ones = np.ones((1, 128), dtype=np.float32)

    in_maps = []
    for d in range(N_CORES):
        shard = np.ascontiguousarray(embs[d * LPC:(d + 1) * LPC].T)
        in_maps.append({
            "embsT": shard, "WxT": WxT, "WlT": WlT, "WrT": WrT,
            "bxr": bxr, "padT": padT, "eye_in": eye, "ones_in": ones,
        })

    nc = _get_nc()
    res = run_bass_kernel_spmd(nc, in_maps, list(range(N_CORES)),
                               trace=_trace, **(_trace_kwargs or {}))
    _CACHE["last_result"] = res
    return np.asarray(res.results[0]["out"], dtype=np.float32)
